# revision 11
# baseline (speedup 1.0000x reference)
"""CARAFE forward on 8 TRN2 NeuronCores — v2 (dense bands, lean traffic).

Problem: features (8,128,64,64) f32, masks (8,25,128,128) f32
         -> out (8,128,128,128) f32, KERNEL_SIZE=5, GROUP=1, SCALE=2.
Sharding: pure data-parallel, one batch sample per core.

Banded-matmul formulation as the baseline: per output row h, columns
(p, w, q) (N=256), contraction over x = w+j-2 with feature-row pairs
stacked along K=128; 3 i-groups {0,1}, {2,3}, {4}.  v2 over baseline:
 1. fp16 device output (4 MiB instead of 8), upcast to f32 on host.
 2. group2 (i=4) band slab ships top-half only (64 partitions, saves
    2.1 MiB); its bottom K-half reads a once-memset zero region.
 3. ft2 and band slabs stream in 16-row chunks, first matmul after
    ~1.5 MiB of DMA instead of ~5 MiB.
 4. h-major matmul order (all of row h's 3-4 matmuls back-to-back) —
    PSUM accumulation groups never interleave within a bank (start=True
    clears the whole bank's has_written bits, so two OPEN groups must
    not share a bank).
Total HBM traffic ~16.7 MiB vs ~22.6 MiB baseline.
"""

import numpy as np

N_CORES = 8
C, H, W = 128, 64, 64
K5 = 5
PAD = 2
KX = W
NCOL = 256                # (p, w, q) output columns per low-res row h
HB = 8                    # h rows per PSUM mega-tile
HC = 16                   # h rows per streamed band chunk
NCHUNK = H // HC          # 4

_compiled = {}


def _build_program(n_reps: int = 1):
    import concourse.bacc as bacc
    import concourse.mybir as mybir
    import concourse.tile as tile

    dt16 = mybir.dt.float16
    nc = bacc.Bacc("TRN2", target_bir_lowering=False, debug=False,
                   num_devices=N_CORES)

    ft2 = nc.dram_tensor("ft2", [2 * W, H, C], dt16, kind="ExternalInput")
    s2f = nc.dram_tensor("s2f", [NCHUNK, 2, 2 * KX, HC, NCOL], dt16,
                         kind="ExternalInput")
    s2g = nc.dram_tensor("s2g", [NCHUNK, KX, HC, NCOL], dt16,
                         kind="ExternalInput")
    out = nc.dram_tensor("out", [C, 2 * H, 2 * W], dt16,
                         kind="ExternalOutput")

    SB = 4                # band-chunk ring depth (= NCHUNK: no reuse, no WAR gating)

    def h_matmuls(h):
        """(feature pair r, group) for output row h, in issue order."""
        mm = []
        if h == 1:
            mm.append((0, 0))          # i=1 alone: top of pair 0
        elif h >= 2:
            mm.append((h - 2, 0))      # i={0,1}: pair h-2
        mm.append((h, 1))              # i={2,3}: pair h
        if h + 2 < H:
            mm.append((h + 2, 2))      # i=4: top of pair h+2
        return mm

    def body(fp, sp, ps, ob, first):
        ft = fp.tile([2 * W, H, C], dt16, tag="ft", name="ft")
        ss = [[sp.tile([C, HC, NCOL], dt16, tag=f"s{g}{b}", name=f"ss{g}{b}")
               for b in range(SB)] for g in range(3)]
        if first:
            # group2 bottom K-half must read zeros; set once, never dirtied
            for b in range(SB):
                nc.gpsimd.memset(ss[2][b][64:128, :, :], 0.0)

        def dma_chunk(k):
            # all inputs on the SP HWDGE ring (ordered, saturated);
            # outputs ride the ACT ring so they never block input chunks
            b = k % SB
            if k == 0:
                # split h-halves so h=0's matmuls unblock after ~0.9 MiB
                for lo, hi in ((0, HC // 2), (HC // 2, HC)):
                    nc.sync.dma_start(ss[1][b][:, lo:hi, :],
                                      s2f[k, 1, :, lo:hi, :])
                    nc.sync.dma_start(ss[2][b][0:64, lo:hi, :],
                                      s2g[k, :, lo:hi, :])
                    nc.sync.dma_start(ss[0][b][:, lo:hi, :],
                                      s2f[k, 0, :, lo:hi, :])
                return
            nc.sync.dma_start(ss[1][b][:], s2f[k, 1])
            nc.sync.dma_start(ss[0][b][:], s2f[k, 0])
            nc.sync.dma_start(ss[2][b][0:64, :, :], s2g[k])

        nc.sync.dma_start(ft[:, 0:HC // 2, :], ft2[:, 0:HC // 2, :])
        dma_chunk(0)
        nc.sync.dma_start(ft[:, HC // 2:HC, :], ft2[:, HC // 2:HC, :])
        for jj in range(1, NCHUNK):
            nc.sync.dma_start(ft[:, HC * jj:HC * (jj + 1), :],
                              ft2[:, HC * jj:HC * (jj + 1), :])
            dma_chunk(jj)

        acc = None
        for h in range(H):
            B = h // HB
            hloc = h % HB
            if hloc == 0:
                acc = ps.tile([C, HB * NCOL], mybir.dt.float32,
                              tag="acc", name=f"acc{B}")
            mms = h_matmuls(h)
            o_sl = acc[:, hloc * NCOL:(hloc + 1) * NCOL]
            for n_i, (r, g) in enumerate(mms):
                rhs = ss[g][(h // HC) % SB][:, h % HC, :]
                nc.tensor.matmul(o_sl, ft[:, r, :], rhs,
                                 start=(n_i == 0), stop=(n_i == len(mms) - 1))
            if hloc == HB - 1:
                o = ob.tile([C, HB * NCOL], dt16, tag="o", name="o")
                nc.vector.tensor_copy(o[:], acc[:])
                nc.scalar.dma_start(
                    out[:, 2 * HB * B:2 * HB * (B + 1), :],
                    o[:].rearrange("c (hp w) -> c hp w", w=2 * W))

    with tile.TileContext(nc) as tc:
        with (
            tc.tile_pool(name="fp", bufs=1) as fp,
            tc.tile_pool(name="sp", bufs=1) as sp,
            tc.tile_pool(name="ps", bufs=2, space="PSUM") as ps,
            tc.tile_pool(name="ob", bufs=3) as ob,
        ):
            for rep in range(n_reps):
                body(fp, sp, ps, ob, first=(rep == 0))

    nc.compile()
    return nc


def _band(masks_n, i):
    """S(h,i) banded matrix for all h: [KX, H, 2, W, 2] from one sample's
    masks [25, 2H, 2W]; S[w+j-2, h, p, w, q] = m[5i+j, 2h+p, 2w+q]."""
    m = masks_n.reshape(K5, K5, H, 2, W, 2)  # [i, j, h, p, w, q]
    s = np.zeros((KX, H, 2, W, 2), dtype=np.float16)
    for j in range(K5):
        wlo = max(0, PAD - j)
        whi = min(W, W + PAD - j)
        wi = np.arange(wlo, whi)
        s[wi + j - PAD, :, :, wi, :] = m[i, j, :, :, wlo:whi].transpose(
            2, 0, 1, 3)
    return s


def _prep_inputs(features: np.ndarray, masks: np.ndarray):
    """Host-side layout prep: stacked ft2, chunked band slabs."""
    n = features.shape[0]
    ftw = features.transpose(0, 3, 2, 1).astype(np.float16)  # [n, w, h, c]
    ft2 = np.zeros((n, 2 * KX, H, C), dtype=np.float16)
    ft2[:, :KX] = ftw
    ft2[:, KX:, :H - 1] = ftw[:, :, 1:]      # row h+1; zero at h = H-1

    # dense slabs per group: sg[n, g, x(128 or 64), h, col]
    s01 = np.zeros((n, 2, 2 * KX, H, NCOL), dtype=np.float16)
    sg2 = np.zeros((n, KX, H, NCOL), dtype=np.float16)
    for smp in range(n):
        bands = [_band(masks[smp], i).reshape(KX, H, NCOL) for i in range(K5)]
        s01[smp, 0, :KX, 2:] = bands[0][:, 2:]     # g0 top: i=0  (h >= 2)
        s01[smp, 0, KX:, 2:] = bands[1][:, 2:]     # g0 bot: i=1  (h >= 2)
        s01[smp, 0, :KX, 1] = bands[1][:, 1]       # h=1: i=1 on top
        s01[smp, 1, :KX, :] = bands[2]             # g1 top: i=2  (all h)
        s01[smp, 1, KX:, :H - 1] = bands[3][:, :H - 1]  # g1 bot: i=3
        sg2[smp, :, :H - 2] = bands[4][:, :H - 2]  # g2 top: i=4  (h <= 61)

    # chunk the h dim: [n, chunk, ...]
    s2f = np.ascontiguousarray(
        s01.reshape(n, 2, 2 * KX, NCHUNK, HC, NCOL).transpose(0, 3, 1, 2, 4, 5))
    s2g = np.ascontiguousarray(
        sg2.reshape(n, KX, NCHUNK, HC, NCOL).transpose(0, 2, 1, 3, 4))
    return ft2, s2f, s2g


def kernel(features: np.ndarray, masks: np.ndarray) -> np.ndarray:
    from concourse.bass_utils import run_bass_kernel_spmd

    if 1 not in _compiled:
        _compiled[1] = _build_program(1)
    nc = _compiled[1]

    ft2, s2f, s2g = _prep_inputs(np.asarray(features, dtype=np.float32),
                                 np.asarray(masks, dtype=np.float32))
    in_maps = [{"ft2": ft2[i], "s2f": s2f[i], "s2g": s2g[i]}
               for i in range(N_CORES)]
    res = run_bass_kernel_spmd(nc, in_maps, list(range(N_CORES)))
    return np.stack([res.results[i]["out"] for i in range(N_CORES)],
                    axis=0).astype(np.float32)


# revision 12
# speedup vs baseline: 1.1145x; 1.1145x over previous
"""CARAFE forward on 8 TRN2 NeuronCores — v2 (dense bands, lean traffic).

Problem: features (8,128,64,64) f32, masks (8,25,128,128) f32
         -> out (8,128,128,128) f32, KERNEL_SIZE=5, GROUP=1, SCALE=2.
Sharding: pure data-parallel, one batch sample per core.

Banded-matmul formulation as the baseline: per output row h, columns
(p, w, q) (N=256), contraction over x = w+j-2 with feature-row pairs
stacked along K=128; 3 i-groups {0,1}, {2,3}, {4}.  v2 over baseline:
 1. fp16 device output (4 MiB instead of 8), upcast to f32 on host.
 2. group2 (i=4) band slab ships top-half only (64 partitions, saves
    2.1 MiB); its bottom K-half reads a once-memset zero region.
 3. ft2 and band slabs stream in 16-row chunks, first matmul after
    ~1.5 MiB of DMA instead of ~5 MiB.
 4. h-major matmul order (all of row h's 3-4 matmuls back-to-back) —
    PSUM accumulation groups never interleave within a bank (start=True
    clears the whole bank's has_written bits, so two OPEN groups must
    not share a bank).
Total HBM traffic ~16.7 MiB vs ~22.6 MiB baseline.
"""

import numpy as np

N_CORES = 8
C, H, W = 128, 64, 64
K5 = 5
PAD = 2
KX = W
NCOL = 256                # (p, w, q) output columns per low-res row h
HB = 8                    # h rows per PSUM mega-tile
HC = 16                   # h rows per streamed band chunk
NCHUNK = H // HC          # 4

_compiled = {}


def _build_program(n_reps: int = 1):
    import concourse.bacc as bacc
    import concourse.mybir as mybir
    import concourse.tile as tile

    dt16 = mybir.dt.float16
    nc = bacc.Bacc("TRN2", target_bir_lowering=False, debug=False,
                   num_devices=N_CORES)

    ft2 = nc.dram_tensor("ft2", [2 * W, H, C], dt16, kind="ExternalInput")
    s2f = nc.dram_tensor("s2f", [NCHUNK, 2, 2 * KX, HC, NCOL], dt16,
                         kind="ExternalInput")
    s2g = nc.dram_tensor("s2g", [NCHUNK, KX, HC, NCOL], dt16,
                         kind="ExternalInput")
    out = nc.dram_tensor("out", [C, 2 * H, 2 * W], dt16,
                         kind="ExternalOutput")

    SB = 4                # band-chunk ring depth (= NCHUNK: no reuse, no WAR gating)

    def h_matmuls(h):
        """(feature pair r, group) for output row h, in issue order."""
        mm = []
        if h == 1:
            mm.append((0, 0))          # i=1 alone: top of pair 0
        elif h >= 2:
            mm.append((h - 2, 0))      # i={0,1}: pair h-2
        mm.append((h, 1))              # i={2,3}: pair h
        if h + 2 < H:
            mm.append((h + 2, 2))      # i=4: top of pair h+2
        return mm

    def body(fp, sp, ps, ob, first):
        ft = fp.tile([2 * W, H, C], dt16, tag="ft", name="ft")
        ss = [[sp.tile([C, HC, NCOL], dt16, tag=f"s{g}{b}", name=f"ss{g}{b}")
               for b in range(SB)] for g in range(3)]
        if first:
            # group2 bottom K-half must read zeros; set once, never dirtied
            for b in range(SB):
                nc.gpsimd.memset(ss[2][b][64:128, :, :], 0.0)

        def dma_chunk(k):
            # all inputs on the SP HWDGE ring (ordered, saturated);
            # outputs ride the ACT ring so they never block input chunks
            b = k % SB
            nc.sync.dma_start(ss[1][b][:], s2f[k, 1])
            nc.sync.dma_start(ss[0][b][:], s2f[k, 0])
            nc.sync.dma_start(ss[2][b][0:64, :, :], s2g[k])

        for jj in range(NCHUNK):
            nc.sync.dma_start(ft[:, HC * jj:HC * (jj + 1), :],
                              ft2[:, HC * jj:HC * (jj + 1), :])
            dma_chunk(jj)

        acc = None
        for h in range(H):
            B = h // HB
            hloc = h % HB
            if hloc == 0:
                acc = ps.tile([C, HB * NCOL], mybir.dt.float32,
                              tag="acc", name=f"acc{B}")
            mms = h_matmuls(h)
            o_sl = acc[:, hloc * NCOL:(hloc + 1) * NCOL]
            for n_i, (r, g) in enumerate(mms):
                rhs = ss[g][(h // HC) % SB][:, h % HC, :]
                nc.tensor.matmul(o_sl, ft[:, r, :], rhs,
                                 start=(n_i == 0), stop=(n_i == len(mms) - 1))
            if hloc == HB - 1:
                o = ob.tile([C, HB * NCOL], dt16, tag="o", name="o")
                nc.vector.tensor_copy(o[:], acc[:])
                nc.scalar.dma_start(
                    out[:, 2 * HB * B:2 * HB * (B + 1), :],
                    o[:].rearrange("c (hp w) -> c hp w", w=2 * W))

    with tile.TileContext(nc) as tc:
        with (
            tc.tile_pool(name="fp", bufs=1) as fp,
            tc.tile_pool(name="sp", bufs=1) as sp,
            tc.tile_pool(name="ps", bufs=2, space="PSUM") as ps,
            tc.tile_pool(name="ob", bufs=3) as ob,
        ):
            for rep in range(n_reps):
                body(fp, sp, ps, ob, first=(rep == 0))

    nc.compile()
    return nc


def _band(masks_n, i):
    """S(h,i) banded matrix for all h: [KX, H, 2, W, 2] from one sample's
    masks [25, 2H, 2W]; S[w+j-2, h, p, w, q] = m[5i+j, 2h+p, 2w+q]."""
    m = masks_n.reshape(K5, K5, H, 2, W, 2)  # [i, j, h, p, w, q]
    s = np.zeros((KX, H, 2, W, 2), dtype=np.float16)
    for j in range(K5):
        wlo = max(0, PAD - j)
        whi = min(W, W + PAD - j)
        wi = np.arange(wlo, whi)
        s[wi + j - PAD, :, :, wi, :] = m[i, j, :, :, wlo:whi].transpose(
            2, 0, 1, 3)
    return s


def _prep_inputs(features: np.ndarray, masks: np.ndarray):
    """Host-side layout prep: stacked ft2, chunked band slabs."""
    n = features.shape[0]
    ftw = features.transpose(0, 3, 2, 1).astype(np.float16)  # [n, w, h, c]
    ft2 = np.zeros((n, 2 * KX, H, C), dtype=np.float16)
    ft2[:, :KX] = ftw
    ft2[:, KX:, :H - 1] = ftw[:, :, 1:]      # row h+1; zero at h = H-1

    # dense slabs per group: sg[n, g, x(128 or 64), h, col]
    s01 = np.zeros((n, 2, 2 * KX, H, NCOL), dtype=np.float16)
    sg2 = np.zeros((n, KX, H, NCOL), dtype=np.float16)
    for smp in range(n):
        bands = [_band(masks[smp], i).reshape(KX, H, NCOL) for i in range(K5)]
        s01[smp, 0, :KX, 2:] = bands[0][:, 2:]     # g0 top: i=0  (h >= 2)
        s01[smp, 0, KX:, 2:] = bands[1][:, 2:]     # g0 bot: i=1  (h >= 2)
        s01[smp, 0, :KX, 1] = bands[1][:, 1]       # h=1: i=1 on top
        s01[smp, 1, :KX, :] = bands[2]             # g1 top: i=2  (all h)
        s01[smp, 1, KX:, :H - 1] = bands[3][:, :H - 1]  # g1 bot: i=3
        sg2[smp, :, :H - 2] = bands[4][:, :H - 2]  # g2 top: i=4  (h <= 61)

    # chunk the h dim: [n, chunk, ...]
    s2f = np.ascontiguousarray(
        s01.reshape(n, 2, 2 * KX, NCHUNK, HC, NCOL).transpose(0, 3, 1, 2, 4, 5))
    s2g = np.ascontiguousarray(
        sg2.reshape(n, KX, NCHUNK, HC, NCOL).transpose(0, 2, 1, 3, 4))
    return ft2, s2f, s2g


def kernel(features: np.ndarray, masks: np.ndarray) -> np.ndarray:
    from concourse.bass_utils import run_bass_kernel_spmd

    if 1 not in _compiled:
        _compiled[1] = _build_program(1)
    nc = _compiled[1]

    ft2, s2f, s2g = _prep_inputs(np.asarray(features, dtype=np.float32),
                                 np.asarray(masks, dtype=np.float32))
    in_maps = [{"ft2": ft2[i], "s2f": s2f[i], "s2g": s2g[i]}
               for i in range(N_CORES)]
    res = run_bass_kernel_spmd(nc, in_maps, list(range(N_CORES)))
    return np.stack([res.results[i]["out"] for i in range(N_CORES)],
                    axis=0).astype(np.float32)
